# revision 28
# baseline (speedup 1.0000x reference)
"""Trainium2 Bass kernel for Autoformer-style autocorrelation attention.

Math (matches the reference nn.Module):
    top_k = int(log(L)) = 6
    mean_value[b, l] = corr[b].mean(over H, C)                     # [B, L]
    idx = top_k(mean_value.mean(over B))                           # [6]
    w = softmax(mean_value[:, idx], axis=-1)                       # [B, 6]
    out[b, h, c, l] = sum_k w[b, k] * values[b, h, c, (l+idx_k)%L]

Strategy: data-parallel over B (4 batches per core on 8 cores).

Launch 1 reduces corr over (H, C) per batch on-device via ones-matmuls
over the partition axis.  corr is sent as fp16: the quantization error on
the means (~1e-5) is far below the 4.8e-4 top-k selection margin measured
on this distribution, and it halves launch-1 HBM traffic.  The [32, L]
sums return to host, where the tiny top-k + softmax glue runs.

Launch 2 bakes the 6 indices in as static SBUF column windows.  values
are sent as fp16 (quantization ~5e-4 relative on the output, fp32
accumulation throughout).  Each [128, L] tile is doubled in SBUF
([v, v], the copy via the idle GpSimd DMA queue) so every shifted window
is a single contiguous 512-wide matmul: 5 shift terms run on the tensor
engine as diag(w_bk) @ window matmuls accumulating in PSUM (diag matrices
are precomputed on host and DMA'd), the 6th shift is one scaled-copy
ACTIVATE on the scalar engine, and the vector engine does the single
(t5 + psum) merge per output half before DMA-out.  Per-batch weights and
diags enter through input tensors so one compiled NEFF is SPMD across
all 8 cores.
"""

import math

import numpy as np

_B, _H, _C, _L = 32, 8, 64, 1024
_NCORES = 8
_BLOC = _B // _NCORES  # batches per core
_R = _H * _C           # rows per batch
_PART = 128
_TPB = _R // _PART     # SBUF tiles per batch
_TOPK = int(math.log(_L))  # 6
_NPE = 5               # shift terms handled by the tensor engine
_HALF = 512            # PSUM bank width in fp32


def _build_phase1():
    import concourse.bacc as bacc
    import concourse.mybir as mybir
    import concourse.tile as tile

    f32 = mybir.dt.float32
    f16 = mybir.dt.float16
    nc = bacc.Bacc("TRN2", target_bir_lowering=False, debug=False,
                   enable_partition_id=False)
    corr_d = nc.dram_tensor("corr_sh", [_BLOC, _R, _L], f16, kind="ExternalInput").ap()
    sums_d = nc.dram_tensor("sums", [1, _BLOC * _L], f32, kind="ExternalOutput").ap()

    with tile.TileContext(nc) as tc:
        with (
            tc.tile_pool(name="io", bufs=6) as io_pool,
            tc.tile_pool(name="const", bufs=1) as const_pool,
            tc.tile_pool(name="acc", bufs=1) as acc_pool,
            tc.tile_pool(name="ps", bufs=3, space="PSUM") as ps_pool,
        ):
            ones = const_pool.tile([_PART, _HALF], f16)
            nc.vector.memset(ones[:], 1.0)
            outs = acc_pool.tile([1, _BLOC * _L], f32)
            # HAM warmup: ~4us of junk matmuls so the PE clock is at 2.4GHz
            # when the real stream starts (overlaps the entry barrier + first
            # DMA latency)
            wps = ps_pool.tile([_PART, _HALF], f32, tag="wps", name="wps", bufs=1)
            for _ in range(8):
                nc.tensor.matmul(wps[:], ones[:, 0:_PART], ones[:],
                                 start=True, stop=True)
            for b in range(_BLOC):
                pss = [ps_pool.tile([_PART, _HALF], f32, tag=f"ps{h}", name=f"ps{h}")
                       for h in range(2)]
                for t2 in range(_TPB // 2):
                    # two row-blocks per DMA: [128, 2L] tile, halving DMA count
                    vt = io_pool.tile([_PART, 2 * _L], f16, tag="vt")
                    src_ap = corr_d[b, t2 * 2 * _PART:(t2 + 1) * 2 * _PART, :]
                    nc.sync.dma_start(
                        vt[:].rearrange("p (u l) -> p u l", u=2),
                        src_ap.rearrange("(u p) l -> p u l", p=_PART))
                    for u in range(2):
                        for h in range(2):
                            nc.tensor.matmul(
                                pss[h][:],
                                ones[:, 0:_PART],
                                vt[:, u * _L + h * _HALF:u * _L + (h + 1) * _HALF],
                                start=(t2 == 0 and u == 0),
                                stop=(t2 == _TPB // 2 - 1 and u == 1),
                            )
                for h in range(2):
                    o0 = b * _L + h * _HALF
                    nc.scalar.copy(outs[0:1, o0:o0 + _HALF], pss[h][0:1, :])
                nc.scalar.dma_start(
                    sums_d[0:1, b * _L:(b + 1) * _L],
                    outs[0:1, b * _L:(b + 1) * _L])
    nc.compile()
    return nc


def _build_phase2(idx):
    import concourse.bacc as bacc
    import concourse.mybir as mybir
    import concourse.tile as tile

    f32 = mybir.dt.float32
    f16 = mybir.dt.float16
    alu = mybir.AluOpType
    act_copy = mybir.ActivationFunctionType.Copy

    nc = bacc.Bacc("TRN2", target_bir_lowering=False, debug=False,
                   enable_partition_id=False)
    vals_d = nc.dram_tensor("vals", [_BLOC, _R, _L], f16, kind="ExternalInput").ap()
    wsb_d = nc.dram_tensor("wsb", [_PART, _BLOC * _TOPK], f32, kind="ExternalInput").ap()
    diag_d = nc.dram_tensor(
        "diags", [_PART, _BLOC * _NPE * _PART], f16, kind="ExternalInput").ap()
    out_d = nc.dram_tensor("out_sh", [_BLOC, _R, _L], f32, kind="ExternalOutput").ap()

    with tile.TileContext(nc) as tc:
        with (
            tc.tile_pool(name="const", bufs=1) as const_pool,
            tc.tile_pool(name="v16", bufs=8) as v16_pool,
            tc.tile_pool(name="tmp", bufs=4) as tmp_pool,
            tc.tile_pool(name="out", bufs=4) as out_pool,
            tc.tile_pool(name="ps", bufs=3, space="PSUM") as ps_pool,
        ):
            # HAM warmup on an independent memset tile: starts right after
            # the entry barrier, overlapping the const/input DMA latency
            wones = const_pool.tile([_PART, _HALF], f16)
            nc.vector.memset(wones[:], 1.0)
            wps = ps_pool.tile([_PART, _HALF], f32, tag="ps0", name="wps", bufs=4)
            for _ in range(22):
                nc.tensor.matmul(wps[:], wones[:, 0:_PART], wones[:],
                                 start=True, stop=True)
            w_t = const_pool.tile([_PART, _BLOC * _TOPK], f32)
            nc.sync.dma_start(w_t[:], wsb_d[:])
            diag = const_pool.tile([_PART, _BLOC * _NPE * _PART], f16)
            nc.sync.dma_start(diag[:], diag_d[:])

            for b in range(_BLOC):
                for t in range(_TPB):
                    vt16 = v16_pool.tile([_PART, _L], f16, tag="vt16")
                    nc.sync.dma_start(
                        vt16[:], vals_d[b, t * _PART:(t + 1) * _PART, :])

                    pss = [ps_pool.tile([_PART, _HALF], f32, tag=f"ps{h}",
                                        name=f"ps{h}", bufs=4)
                           for h in range(2)]
                    # wrap-split pieces; start only on the FIRST matmul into a
                    # bank (it clears the whole bank's has_written bits), stop
                    # on the last.
                    pieces = {0: [], 1: []}
                    for k in range(_NPE):
                        dof = (b * _NPE + k) * _PART
                        for h in range(2):
                            s = (idx[k] + h * _HALF) % _L
                            n1 = min(_HALF, _L - s)
                            pieces[h].append((dof, 0, n1, s))
                            if n1 < _HALF:
                                pieces[h].append((dof, n1, _HALF, 0))
                    for h in range(2):
                        for pi, (dof, o0, o1, s) in enumerate(pieces[h]):
                            nc.tensor.matmul(
                                pss[h][:, o0:o1], diag[:, dof:dof + _PART],
                                vt16[:, s:s + (o1 - o0)],
                                start=(pi == 0), stop=(pi == len(pieces[h]) - 1),
                            )

                    # shift term 5 on the scalar engine: t5 = w5 * roll(v)
                    t5 = tmp_pool.tile([_PART, _L], f32, tag="t5")
                    s5 = idx[_NPE]
                    w5 = w_t[:, b * _TOPK + _NPE:b * _TOPK + _NPE + 1]
                    if s5 == 0:
                        nc.scalar.activation(t5[:], vt16[:], act_copy, scale=w5)
                    else:
                        nc.scalar.activation(
                            t5[:, 0:_L - s5], vt16[:, s5:_L], act_copy, scale=w5)
                        nc.scalar.activation(
                            t5[:, _L - s5:_L], vt16[:, 0:s5], act_copy, scale=w5)

                    ot = out_pool.tile([_PART, _L], f32, tag="ot")
                    for h in range(2):
                        nc.vector.scalar_tensor_tensor(
                            ot[:, h * _HALF:(h + 1) * _HALF],
                            t5[:, h * _HALF:(h + 1) * _HALF],
                            1.0,
                            pss[h][:],
                            op0=alu.mult,
                            op1=alu.add,
                        )
                    nc.scalar.dma_start(out_d[b, t * _PART:(t + 1) * _PART, :], ot[:])
    nc.compile()
    return nc


def _run_spmd(nc, in_maps, **kwargs):
    from concourse import bass_utils

    return bass_utils.run_bass_kernel_spmd(
        nc, in_maps, core_ids=list(range(_NCORES)), **kwargs
    )


def kernel(values: np.ndarray, corr: np.ndarray, _collect=None) -> np.ndarray:
    assert values.shape == (_B, _H, _C, _L) and corr.shape == (_B, _H, _C, _L)
    corr16 = np.ascontiguousarray(
        np.asarray(corr, dtype=np.float32).reshape(_B, _R, _L), dtype=np.float16
    )
    vals16 = np.ascontiguousarray(
        np.asarray(values, dtype=np.float32).reshape(_B, _R, _L), dtype=np.float16
    )

    # ---- launch 1: per-batch sums of corr over (H, C) ----
    nc1 = _build_phase1()
    in1 = [
        {"corr_sh": corr16[c * _BLOC:(c + 1) * _BLOC]}
        for c in range(_NCORES)
    ]
    res1 = _run_spmd(nc1, in1, **(_collect.kwargs(1) if _collect else {}))
    if _collect is not None:
        _collect.add(1, nc1, res1)
    sums = np.concatenate(
        [r["sums"].reshape(_BLOC, _L) for r in res1.results], axis=0
    )  # [B, L]

    # ---- host glue: top-k indices + softmax weights (tiny) ----
    mean_value = sums / np.float32(_R)                       # [B, L]
    g = mean_value.astype(np.float64).mean(axis=0)           # [L]
    idx = np.argsort(-g, kind="stable")[:_TOPK].astype(np.int64)
    wsel = mean_value[:, idx].astype(np.float32)             # [B, 6]
    e = np.exp(wsel - wsel.max(axis=-1, keepdims=True))
    w = (e / e.sum(axis=-1, keepdims=True)).astype(np.float32)

    # ---- launch 2: weighted shifted-gather combine ----
    nc2 = _build_phase2([int(i) for i in idx])
    eye = np.eye(_PART, dtype=np.float16)
    in2 = []
    for c in range(_NCORES):
        wloc = w[c * _BLOC:(c + 1) * _BLOC]                  # [BLOC, 6]
        wsb = np.ascontiguousarray(
            np.broadcast_to(wloc.reshape(-1)[None, :], (_PART, _BLOC * _TOPK)),
            dtype=np.float32,
        )
        diags = np.concatenate(
            [eye * np.float16(wloc[b, k]) for b in range(_BLOC)
             for k in range(_NPE)],
            axis=1,
        )  # [128, BLOC*NPE*128] fp16
        in2.append({
            "vals": vals16[c * _BLOC:(c + 1) * _BLOC],
            "wsb": wsb,
            "diags": np.ascontiguousarray(diags),
        })
    res2 = _run_spmd(nc2, in2, **(_collect.kwargs(2) if _collect else {}))
    if _collect is not None:
        _collect.add(2, nc2, res2)
    out = np.concatenate([r["out_sh"] for r in res2.results], axis=0)
    return out.reshape(_B, _H, _C, _L)


# revision 29
# speedup vs baseline: 1.0203x; 1.0203x over previous
"""Trainium2 Bass kernel for Autoformer-style autocorrelation attention.

Math (matches the reference nn.Module):
    top_k = int(log(L)) = 6
    mean_value[b, l] = corr[b].mean(over H, C)                     # [B, L]
    idx = top_k(mean_value.mean(over B))                           # [6]
    w = softmax(mean_value[:, idx], axis=-1)                       # [B, 6]
    out[b, h, c, l] = sum_k w[b, k] * values[b, h, c, (l+idx_k)%L]

Strategy: data-parallel over B (4 batches per core on 8 cores).

Launch 1 reduces corr over (H, C) per batch on-device via ones-matmuls
over the partition axis.  corr is sent as fp16: the quantization error on
the means (~1e-5) is far below the 4.8e-4 top-k selection margin measured
on this distribution, and it halves launch-1 HBM traffic.  The [32, L]
sums return to host, where the tiny top-k + softmax glue runs.

Launch 2 bakes the 6 indices in as static SBUF column windows.  values
are sent as fp16 (quantization ~5e-4 relative on the output, fp32
accumulation throughout).  Each [128, L] tile is doubled in SBUF
([v, v], the copy via the idle GpSimd DMA queue) so every shifted window
is a single contiguous 512-wide matmul: 5 shift terms run on the tensor
engine as diag(w_bk) @ window matmuls accumulating in PSUM (diag matrices
are precomputed on host and DMA'd), the 6th shift is one scaled-copy
ACTIVATE on the scalar engine, and the vector engine does the single
(t5 + psum) merge per output half before DMA-out.  Per-batch weights and
diags enter through input tensors so one compiled NEFF is SPMD across
all 8 cores.
"""

import math

import numpy as np

_B, _H, _C, _L = 32, 8, 64, 1024
_NCORES = 8
_BLOC = _B // _NCORES  # batches per core
_R = _H * _C           # rows per batch
_PART = 128
_TPB = _R // _PART     # SBUF tiles per batch
_TOPK = int(math.log(_L))  # 6
_NPE = 5               # shift terms handled by the tensor engine
_HALF = 512            # PSUM bank width in fp32


def _build_phase1():
    import concourse.bacc as bacc
    import concourse.mybir as mybir
    import concourse.tile as tile

    f32 = mybir.dt.float32
    f16 = mybir.dt.float16
    nc = bacc.Bacc("TRN2", target_bir_lowering=False, debug=False,
                   enable_partition_id=False)
    corr_d = nc.dram_tensor("corr_sh", [_BLOC, _R, _L], f16, kind="ExternalInput").ap()
    sums_d = nc.dram_tensor("sums", [1, _BLOC * _L], f32, kind="ExternalOutput").ap()

    with tile.TileContext(nc) as tc:
        with (
            tc.tile_pool(name="io", bufs=6) as io_pool,
            tc.tile_pool(name="const", bufs=1) as const_pool,
            tc.tile_pool(name="acc", bufs=1) as acc_pool,
            tc.tile_pool(name="ps", bufs=3, space="PSUM") as ps_pool,
        ):
            ones = const_pool.tile([_PART, _HALF], f16)
            nc.vector.memset(ones[:], 1.0)
            outs = acc_pool.tile([1, _BLOC * _L], f32)
            # HAM warmup: ~4us of junk matmuls so the PE clock is at 2.4GHz
            # when the real stream starts (overlaps the entry barrier + first
            # DMA latency)
            wps = ps_pool.tile([_PART, _HALF], f32, tag="wps", name="wps", bufs=1)
            for _ in range(8):
                nc.tensor.matmul(wps[:], ones[:, 0:_PART], ones[:],
                                 start=True, stop=True)
            for b in range(_BLOC):
                pss = [ps_pool.tile([_PART, _HALF], f32, tag=f"ps{h}", name=f"ps{h}")
                       for h in range(2)]
                for t2 in range(_TPB // 2):
                    # two row-blocks per DMA: [128, 2L] tile, halving DMA count
                    vt = io_pool.tile([_PART, 2 * _L], f16, tag="vt")
                    src_ap = corr_d[b, t2 * 2 * _PART:(t2 + 1) * 2 * _PART, :]
                    nc.sync.dma_start(
                        vt[:].rearrange("p (u l) -> p u l", u=2),
                        src_ap.rearrange("(u p) l -> p u l", p=_PART))
                    for u in range(2):
                        for h in range(2):
                            nc.tensor.matmul(
                                pss[h][:],
                                ones[:, 0:_PART],
                                vt[:, u * _L + h * _HALF:u * _L + (h + 1) * _HALF],
                                start=(t2 == 0 and u == 0),
                                stop=(t2 == _TPB // 2 - 1 and u == 1),
                            )
                for h in range(2):
                    o0 = b * _L + h * _HALF
                    nc.scalar.copy(outs[0:1, o0:o0 + _HALF], pss[h][0:1, :])
                nc.scalar.dma_start(
                    sums_d[0:1, b * _L:(b + 1) * _L],
                    outs[0:1, b * _L:(b + 1) * _L])
    nc.compile()
    return nc


def _build_phase2(idx):
    import concourse.bacc as bacc
    import concourse.mybir as mybir
    import concourse.tile as tile

    f32 = mybir.dt.float32
    f16 = mybir.dt.float16
    alu = mybir.AluOpType
    act_copy = mybir.ActivationFunctionType.Copy

    nc = bacc.Bacc("TRN2", target_bir_lowering=False, debug=False,
                   enable_partition_id=False)
    vals_d = nc.dram_tensor("vals", [_BLOC, _R, _L], f16, kind="ExternalInput").ap()
    wsb_d = nc.dram_tensor("wsb", [_PART, _BLOC * _TOPK], f32, kind="ExternalInput").ap()
    diag_d = nc.dram_tensor(
        "diags", [_PART, _BLOC * _NPE * _PART], f16, kind="ExternalInput").ap()
    out_d = nc.dram_tensor("out_sh", [_BLOC, _R, _L], f32, kind="ExternalOutput").ap()

    with tile.TileContext(nc) as tc:
        with (
            tc.tile_pool(name="const", bufs=1) as const_pool,
            tc.tile_pool(name="v16", bufs=8) as v16_pool,
            tc.tile_pool(name="tmp", bufs=4) as tmp_pool,
            tc.tile_pool(name="out", bufs=4) as out_pool,
            tc.tile_pool(name="ps", bufs=3, space="PSUM") as ps_pool,
        ):
            # HAM warmup on an independent memset tile: starts right after
            # the entry barrier, overlapping the const/input DMA latency
            wones = const_pool.tile([_PART, _HALF], f16)
            nc.vector.memset(wones[:], 1.0)
            wps = ps_pool.tile([_PART, _HALF], f32, tag="ps0", name="wps", bufs=4)
            for _ in range(22):
                nc.tensor.matmul(wps[:], wones[:, 0:_PART], wones[:],
                                 start=True, stop=True)
            w_t = const_pool.tile([_PART, _BLOC * _TOPK], f32)
            nc.sync.dma_start(w_t[:], wsb_d[:])
            diag = const_pool.tile([_PART, _BLOC * _NPE * _PART], f16)
            nc.sync.dma_start(diag[:], diag_d[:])

            for b in range(_BLOC):
                for t in range(_TPB):
                    vt16 = v16_pool.tile([_PART, _L], f16, tag="vt16")
                    nc.sync.dma_start(
                        vt16[:], vals_d[b, t * _PART:(t + 1) * _PART, :])

                    pss = [ps_pool.tile([_PART, _HALF], f32, tag=f"ps{h}",
                                        name=f"ps{h}", bufs=4)
                           for h in range(2)]
                    # wrap-split pieces; start only on the FIRST matmul into a
                    # bank (it clears the whole bank's has_written bits), stop
                    # on the last.
                    pieces = {0: [], 1: []}
                    for k in range(_NPE):
                        dof = (b * _NPE + k) * _PART
                        for h in range(2):
                            s = (idx[k] + h * _HALF) % _L
                            n1 = min(_HALF, _L - s)
                            pieces[h].append((dof, h, 0, n1, s))
                            if n1 < _HALF:
                                pieces[h].append((dof, h, n1, _HALF, 0))
                    # emit k-major (same diag adjacent across both banks) so
                    # each LDWEIGHTS has a long prefetch window; start/stop
                    # flags stay keyed to per-bank position
                    flat = []
                    for h in range(2):
                        n = len(pieces[h])
                        for pi, p in enumerate(pieces[h]):
                            flat.append((p, pi == 0, pi == n - 1))
                    flat.sort(key=lambda e: e[0][0])  # stable: by diag offset
                    for (dof, h, o0, o1, s), st, sp in flat:
                        nc.tensor.matmul(
                            pss[h][:, o0:o1], diag[:, dof:dof + _PART],
                            vt16[:, s:s + (o1 - o0)],
                            start=st, stop=sp,
                        )

                    # shift term 5 on the scalar engine: t5 = w5 * roll(v)
                    t5 = tmp_pool.tile([_PART, _L], f32, tag="t5")
                    s5 = idx[_NPE]
                    w5 = w_t[:, b * _TOPK + _NPE:b * _TOPK + _NPE + 1]
                    if s5 == 0:
                        nc.scalar.activation(t5[:], vt16[:], act_copy, scale=w5)
                    else:
                        nc.scalar.activation(
                            t5[:, 0:_L - s5], vt16[:, s5:_L], act_copy, scale=w5)
                        nc.scalar.activation(
                            t5[:, _L - s5:_L], vt16[:, 0:s5], act_copy, scale=w5)

                    ot = out_pool.tile([_PART, _L], f32, tag="ot")
                    for h in range(2):
                        nc.vector.scalar_tensor_tensor(
                            ot[:, h * _HALF:(h + 1) * _HALF],
                            t5[:, h * _HALF:(h + 1) * _HALF],
                            1.0,
                            pss[h][:],
                            op0=alu.mult,
                            op1=alu.add,
                        )
                    nc.scalar.dma_start(out_d[b, t * _PART:(t + 1) * _PART, :], ot[:])
    nc.compile()
    return nc


def _run_spmd(nc, in_maps, **kwargs):
    from concourse import bass_utils

    return bass_utils.run_bass_kernel_spmd(
        nc, in_maps, core_ids=list(range(_NCORES)), **kwargs
    )


def kernel(values: np.ndarray, corr: np.ndarray, _collect=None) -> np.ndarray:
    assert values.shape == (_B, _H, _C, _L) and corr.shape == (_B, _H, _C, _L)
    corr16 = np.ascontiguousarray(
        np.asarray(corr, dtype=np.float32).reshape(_B, _R, _L), dtype=np.float16
    )
    vals16 = np.ascontiguousarray(
        np.asarray(values, dtype=np.float32).reshape(_B, _R, _L), dtype=np.float16
    )

    # ---- launch 1: per-batch sums of corr over (H, C) ----
    nc1 = _build_phase1()
    in1 = [
        {"corr_sh": corr16[c * _BLOC:(c + 1) * _BLOC]}
        for c in range(_NCORES)
    ]
    res1 = _run_spmd(nc1, in1, **(_collect.kwargs(1) if _collect else {}))
    if _collect is not None:
        _collect.add(1, nc1, res1)
    sums = np.concatenate(
        [r["sums"].reshape(_BLOC, _L) for r in res1.results], axis=0
    )  # [B, L]

    # ---- host glue: top-k indices + softmax weights (tiny) ----
    mean_value = sums / np.float32(_R)                       # [B, L]
    g = mean_value.astype(np.float64).mean(axis=0)           # [L]
    idx = np.argsort(-g, kind="stable")[:_TOPK].astype(np.int64)
    wsel = mean_value[:, idx].astype(np.float32)             # [B, 6]
    e = np.exp(wsel - wsel.max(axis=-1, keepdims=True))
    w = (e / e.sum(axis=-1, keepdims=True)).astype(np.float32)

    # ---- launch 2: weighted shifted-gather combine ----
    nc2 = _build_phase2([int(i) for i in idx])
    eye = np.eye(_PART, dtype=np.float16)
    in2 = []
    for c in range(_NCORES):
        wloc = w[c * _BLOC:(c + 1) * _BLOC]                  # [BLOC, 6]
        wsb = np.ascontiguousarray(
            np.broadcast_to(wloc.reshape(-1)[None, :], (_PART, _BLOC * _TOPK)),
            dtype=np.float32,
        )
        diags = np.concatenate(
            [eye * np.float16(wloc[b, k]) for b in range(_BLOC)
             for k in range(_NPE)],
            axis=1,
        )  # [128, BLOC*NPE*128] fp16
        in2.append({
            "vals": vals16[c * _BLOC:(c + 1) * _BLOC],
            "wsb": wsb,
            "diags": np.ascontiguousarray(diags),
        })
    res2 = _run_spmd(nc2, in2, **(_collect.kwargs(2) if _collect else {}))
    if _collect is not None:
        _collect.add(2, nc2, res2)
    out = np.concatenate([r["out_sh"] for r in res2.results], axis=0)
    return out.reshape(_B, _H, _C, _L)
